# revision 35
# baseline (speedup 1.0000x reference)
"""AdaptiveMarginLoss distributed Trainium2 kernel (8 NeuronCores, classifier-parallel).

Self-contained: builds + runs a Bass/Tile SPMD kernel over 8 cores.
Shards the class dimension C (padded 10000 -> 10240 = 8 * 1280).

Per core c (CL = 1280 classes):
  fea_c   = l2norm_rows(x) @ l2norm_rows(w_c).T                  [1024, 1280]
  S_c[i]  = sum_j exp(G*(fea_c[i,j] - sp[i] + B*cw_c[j]))        (+cw via K=1 matmul)
  mx_c[i] = max_j!=label masked scores (diag ~1.0 killed by relu threshold)
  AllGather(S_c - corr/8, mx_c) -> global logsumexp / max -> loss (on device)
All l2 normalization on device; rsqrt computed as exp(-0.5*ln(ssq)) so every
scalar-engine op lives in one ACT table set (no LoadActFuncSet thrash).
"""
import sys, os
sys.path.insert(0, "/opt/trn_rl_repo")
import numpy as np

import concourse.bass as bass
from concourse import bacc
import concourse.mybir as mybir
import concourse.tile as tile
from concourse.bass_utils import run_bass_kernel_spmd

B, D, C = 1024, 512, 10000
CP = 10240          # padded class count
CL = CP // 8        # classes per core
KC = D // 128       # contraction chunks (4)
BT = B // 128       # batch tiles (8)
GAMMA, BETA, LAMDA = 9.6, 0.83, 10.0
NCHUNKS = [(0, 512), (512, 512), (1024, 256)]
CORES = list(range(8))
F32 = mybir.dt.float32
F32R = mybir.dt.float32r
AF = mybir.ActivationFunctionType
OP = mybir.AluOpType


def _build(repeat=1):
    nc = bacc.Bacc()
    AXX = mybir.AxisListType.X
    xT_d = nc.declare_dram_parameter("xT", [D, B], F32, isOutput=False)
    wT_d = nc.declare_dram_parameter("wT", [D, CL], F32, isOutput=False)
    wlT_d = nc.declare_dram_parameter("wlT", [D, B], F32, isOutput=False)
    cw_d = nc.declare_dram_parameter("cw", [1, CL], F32, isOutput=False)
    cwl_d = nc.declare_dram_parameter("cwl", [B], F32, isOutput=False)
    id_d = nc.declare_dram_parameter("identb", [128, B], F32, isOutput=False)

    fea_d = nc.declare_dram_parameter("fea", [B, CL], F32, isOutput=True)
    loss_d = nc.declare_dram_parameter("loss", [1, 1], F32, isOutput=True)

    cc_in1 = nc.dram_tensor("cc_in1", [B], F32)
    cc_out1 = nc.dram_tensor("cc_out1", [8, B], F32, addr_space="Shared")
    cc_in2 = nc.dram_tensor("cc_in2", [B], F32)
    cc_out2 = nc.dram_tensor("cc_out2", [8, B], F32, addr_space="Shared")

    with tile.TileContext(nc) as tc:
        with (
            tc.tile_pool(name="big", bufs=1) as big,
            tc.tile_pool(name="work", bufs=2) as work,
            tc.tile_pool(name="small", bufs=1) as small,
            tc.tile_pool(name="mm", bufs=2, space="PSUM") as mmp,
            tc.tile_pool(name="aux", bufs=1, space="PSUM") as auxp,
        ):
            # ---------- loads: spread queues (per-engine HWDGE), chunked ----------
            xT_sb = big.tile([128, KC, B], F32)
            wT_sb = big.tile([128, KC, CL], F32)
            wlT_sb = big.tile([128, KC, B], F32)
            xT_r = xT_d.rearrange("(k p) b -> p k b", p=128)
            wT_r = wT_d.rearrange("(k p) c -> p k c", p=128)
            wlT_r = wlT_d.rearrange("(k p) b -> p k b", p=128)
            for k in range(KC):
                (nc.sync if k % 2 == 0 else nc.gpsimd).dma_start(out=wT_sb[:, k, :], in_=wT_r[:, k, :])
            for k in range(KC):
                nc.sync.dma_start(out=xT_sb[:, k, :], in_=xT_r[:, k, :])
                nc.gpsimd.dma_start(out=wlT_sb[:, k, :], in_=wlT_r[:, k, :])
            cw_row = small.tile([1, CL], F32)
            nc.gpsimd.dma_start(out=cw_row, in_=cw_d[:, :])
            cwl_pm = small.tile([128, BT], F32)
            nc.gpsimd.dma_start(out=cwl_pm, in_=cwl_d.rearrange("(t p) -> p t", p=128))
            identb = small.tile([128, B], F32)
            nc.gpsimd.dma_start(out=identb, in_=id_d[:, :])

            ones_f = small.tile([128, 128], F32)
            nc.vector.memset(ones_f, 1.0)
            ones_r = small.tile([128, 128], F32R)
            nc.vector.tensor_copy(ones_r, ones_f)
            ones1_f = small.tile([1, 128], F32)
            nc.vector.memset(ones1_f, 1.0)
            ones1_r = small.tile([1, 128], F32R)
            nc.vector.tensor_copy(ones1_r, ones1_f)
            ln8neg = small.tile([128, 1], F32)
            nc.vector.memset(ln8neg, -float(np.log(8.0)))
            relu_bias = small.tile([128, 1], F32)
            nc.vector.memset(relu_bias, -0.999e4)
            half = small.tile([1, 1], F32)
            nc.vector.memset(half, 0.5)

            for _rep in range(repeat):
                # ---------- normalize: ssq via ones-matmul; rs = exp(-0.5 ln(ssq)) ----------
                def normalize(src, n, sq_engine, mul_engine, nm):
                    dst = big.tile([128, KC, n], F32R, tag=f"norm_{nm}", bufs=1)
                    if nm == "w":
                        ssq = mmp.tile([128, n], F32, tag="mm", name=f"ssq_{nm}")
                    else:
                        ssq = auxp.tile([128, n], F32, tag="aux", name=f"ssq_{nm}")
                    for k in range(KC):
                        sq = work.tile([128, n], F32R, tag="sq", name=f"sq_{nm}_{k}")
                        sq_engine(out=sq, in0=src[:, k, :], in1=src[:, k, :])
                        for n0 in range(0, n, 512):
                            ns = min(512, n - n0)
                            nc.tensor.matmul(ssq[:, n0:n0 + ns], ones_r, sq[:, n0:n0 + ns],
                                             start=(k == 0), stop=(k == KC - 1))
                    lns = work.tile([128, n], F32, tag="rs", name=f"lns_{nm}")
                    nc.scalar.activation(out=lns, in_=ssq, func=AF.Ln)
                    rs = work.tile([128, n], F32, tag="rs", name=f"rs_{nm}")
                    nc.scalar.activation(out=rs, in_=lns, func=AF.Exp, scale=-0.5)
                    _rs_cache[nm] = rs
                    for k in range(KC):
                        mul_engine(out=dst[:, k, :], in0=src[:, k, :], in1=rs)
                    return dst
                _rs_cache = {}

                def dve_mul(out, in0, in1):
                    nc.vector.tensor_mul(out, in0, in1)

                def gps_mul(out, in0, in1):
                    nc.gpsimd.tensor_mul(out, in0, in1)

                def act_sq(out, in0, in1):
                    nc.scalar.activation(out=out, in_=in0, func=AF.Square)

                wTn = normalize(wT_sb, CL, dve_mul, dve_mul, "w")
                xTn = normalize(xT_sb, B, dve_mul, dve_mul, "x")
                # raw wl: cast to f32r; scores get 1/||wl|| via relu-scale + max post-scale
                wlT_r8 = big.tile([128, KC, B], F32R, tag="norm_wl", bufs=1)
                for k in range(KC):
                    nc.gpsimd.tensor_copy(wlT_r8[:, k, :], wlT_sb[:, k, :])
                # ssq_wl (column sums of wl^2) -> diag -> al_all = rsqrt
                ssq_wl = auxp.tile([128, B], F32, tag="aux", name="ssq_wl")
                for k in range(KC):
                    sqwl = work.tile([128, B], F32R, tag="sq", name=f"sq_wl_{k}")
                    nc.scalar.activation(out=sqwl, in_=wlT_sb[:, k, :], func=AF.Square)
                    for n0 in range(0, B, 512):
                        nc.tensor.matmul(ssq_wl[:, n0:n0 + 512], ones_r, sqwl[:, n0:n0 + 512],
                                         start=(k == 0), stop=(k == KC - 1))
                dgwl = work.tile([128, B], F32, tag="prod", name="dgwl")
                nc.vector.tensor_mul(dgwl, ssq_wl, identb)
                ssq_wl_diag = small.tile([128, BT], F32)
                nc.vector.tensor_reduce(out=ssq_wl_diag,
                                        in_=dgwl.rearrange("p (t i) -> p t i", i=128),
                                        axis=AXX, op=OP.add)
                al_ln = small.tile([128, BT], F32)
                nc.scalar.activation(out=al_ln, in_=ssq_wl_diag, func=AF.Ln)
                al_all = small.tile([128, BT], F32)
                nc.scalar.activation(out=al_all, in_=al_ln, func=AF.Exp, scale=-0.5)
                al14_all = small.tile([128, BT], F32)
                nc.vector.tensor_scalar_mul(al14_all, al_all, 1e4)
                # a_all = diag of rs_x broadcast (rs_x still live from normalize)
                dga = work.tile([128, B], F32, tag="prod", name="dga")
                nc.vector.tensor_mul(dga, _rs_cache["x"], identb)
                a_all = small.tile([128, BT], F32)
                nc.vector.tensor_reduce(out=a_all,
                                        in_=dga.rearrange("p (t i) -> p t i", i=128),
                                        axis=AXX, op=OP.add)

                # ---------- sp (raw rowdot, scaled by a*al afterwards) ----------
                sp_ps = auxp.tile([128, B], F32, tag="aux", name="sp_ps")
                for k in range(KC):
                    prod = work.tile([128, B], F32R, tag="prod")
                    nc.gpsimd.tensor_mul(prod, xT_sb[:, k, :], wlT_sb[:, k, :])
                    for n0 in range(0, B, 512):
                        nc.tensor.matmul(sp_ps[:, n0:n0 + 512], ones_r, prod[:, n0:n0 + 512],
                                         start=(k == 0), stop=(k == KC - 1))
                dgm = work.tile([128, B], F32, tag="prod", name="dgm")
                nc.vector.tensor_mul(dgm, sp_ps, identb)
                bias_all = small.tile([128, BT], F32)
                nc.vector.tensor_reduce(out=bias_all,
                                        in_=dgm.rearrange("p (t i) -> p t i", i=128),
                                        axis=AXX, op=OP.add)
                nc.vector.tensor_mul(bias_all, bias_all, a_all)
                nc.vector.tensor_mul(bias_all, bias_all, al_all)
                nc.vector.tensor_scalar_mul(bias_all, bias_all, -GAMMA)

                corr8 = small.tile([128, BT], F32)
                nc.scalar.activation(out=corr8, in_=cwl_pm, func=AF.Exp,
                                     scale=GAMMA * BETA, bias=ln8neg)

                # ---------- kexp = exp(G*B*cw) broadcast [128, CL] ----------
                kexp_row = small.tile([1, CL], F32R)
                nc.scalar.activation(out=kexp_row, in_=cw_row, func=AF.Exp, scale=GAMMA * BETA)
                kexp_ps = mmp.tile([128, CL], F32, tag="mm", name="kexp_ps")
                for n0 in range(0, CL, 512):
                    ns = min(512, CL - n0)
                    nc.tensor.matmul(kexp_ps[:, n0:n0 + ns], ones1_r, kexp_row[:, n0:n0 + ns],
                                     start=True, stop=True)
                kexp = big.tile([128, CL], F32, tag="kexp", bufs=1)
                nc.scalar.copy(kexp, kexp_ps)

                # ---------- main loop over batch tiles ----------
                HT = BT // 2
                S_h = [small.tile([128, HT], F32, name=f"S_h{h}") for h in range(2)]
                mx_h = [small.tile([128, HT], F32, name=f"mx_h{h}") for h in range(2)]
                for t in range(BT):
                    bsl = slice(t * 128, (t + 1) * 128)
                    pf = mmp.tile([128, CL], F32, tag="mm", name=f"pf{t}")
                    for k in range(KC):
                        for (n0, ns) in NCHUNKS:
                            nc.tensor.matmul(pf[:, n0:n0 + ns], xTn[:, k, bsl],
                                             wTn[:, k, n0:n0 + ns],
                                             start=(k == 0), stop=(k == KC - 1))
                    fea_sb = work.tile([128, CL], F32, tag="fea")
                    if t < 4:
                        nc.vector.tensor_copy(fea_sb, pf)
                    else:
                        nc.scalar.copy(fea_sb, pf)
                    nc.sync.dma_start(out=fea_d[bsl, :], in_=fea_sb)
                    # scores
                    ps_ = mmp.tile([128, CL], F32, tag="mm", name=f"ps{t}")
                    for k in range(KC):
                        for (n0, ns) in NCHUNKS:
                            nc.tensor.matmul(ps_[:, n0:n0 + ns], wlT_r8[:, k, bsl],
                                             wTn[:, k, n0:n0 + ns],
                                             start=(k == 0), stop=(k == KC - 1))
                    r2 = work.tile([128, CL], F32, tag="r2")
                    nc.scalar.activation(out=r2, in_=ps_, func=AF.Relu, scale=al14_all[:, t:t + 1], bias=relu_bias)
                    msk = work.tile([128, CL], F32, tag="msk")
                    nc.vector.scalar_tensor_tensor(out=msk, in0=r2, scalar=-1e9, in1=ps_,
                                                   op0=OP.mult, op1=OP.add)
                    nc.vector.tensor_reduce(out=mx_h[t // 4][:, t % 4:t % 4 + 1], in_=msk,
                                            axis=AXX, op=OP.max)
                    E = work.tile([128, CL], F32, tag="E")
                    nc.scalar.activation(out=E, in_=pf, func=AF.Exp,
                                         bias=bias_all[:, t:t + 1], scale=GAMMA)
                    Ek = work.tile([128, CL], F32, tag="Ek")
                    nc.vector.scalar_tensor_tensor(out=Ek, in0=E, scalar=1.0, in1=kexp,
                                                   op0=OP.mult, op1=OP.mult,
                                                   accum_out=S_h[t // 4][:, t % 4:t % 4 + 1])

                # ---------- split collectives: one AllGather per half ----------
                if _rep == repeat - 1:
                    HB = HT * 128
                    red_halves = []
                    for h, (ci, co) in enumerate(((cc_in1, cc_out1), (cc_in2, cc_out2))):
                        hsl = slice(h * HT, (h + 1) * HT)
                        nc.vector.tensor_mul(mx_h[h], mx_h[h], al_all[:, hsl])
                        S_adj = small.tile([128, HT], F32, name=f"S_adj{h}")
                        nc.vector.tensor_sub(S_adj, S_h[h], corr8[:, hsl])
                        nc.sync.dma_start(out=ci[0:HB].rearrange("(t p) -> p t", p=128), in_=S_adj)
                        nc.gpsimd.dma_start(out=ci[HB:2 * HB].rearrange("(t p) -> p t", p=128), in_=mx_h[h])
                        nc.gpsimd.collective_compute(
                            "AllGather", OP.bypass,
                            ins=[ci[:]], outs=[co[:, :]],
                            replica_groups=[CORES],
                        )
                        gS = small.tile([128, HT, 8], F32, name=f"gS{h}")
                        gM = small.tile([128, HT, 8], F32, name=f"gM{h}")
                        engs = [nc.sync, nc.gpsimd]
                        for t in range(HT):
                            engs[t % 2].dma_start(out=gS[:, t, :], in_=co[:, t * 128:(t + 1) * 128].rearrange("r p -> p r"))
                            engs[(t + 1) % 2].dma_start(out=gM[:, t, :], in_=co[:, HB + t * 128:HB + (t + 1) * 128].rearrange("r p -> p r"))
                        S_tot = small.tile([128, HT], F32, name=f"S_tot{h}")
                        nc.vector.tensor_reduce(out=S_tot, in_=gS, axis=AXX, op=OP.add)
                        mx_tot = small.tile([128, HT], F32, name=f"mx_tot{h}")
                        nc.vector.tensor_reduce(out=mx_tot, in_=gM, axis=AXX, op=OP.max)
                        loss1 = small.tile([128, HT], F32, name=f"loss1{h}")
                        nc.scalar.activation(out=loss1, in_=S_tot, func=AF.Ln, bias=1.0)
                        redh = small.tile([128, 2], F32, name=f"redh{h}")
                        nc.vector.tensor_reduce(out=redh[:, 0:1], in_=loss1, axis=AXX, op=OP.add)
                        nc.vector.tensor_reduce(out=redh[:, 1:2], in_=mx_tot, axis=AXX, op=OP.add)
                        red_halves.append(redh)
                    red = small.tile([128, 2], F32)
                    nc.vector.tensor_add(red, red_halves[0], red_halves[1])
                    rowt = small.tile([1, 2, 128], F32)
                    nc.gpsimd.dma_start(out=rowt[0:1, 0, :], in_=red[:, 0:1])
                    nc.sync.dma_start(out=rowt[0:1, 1, :], in_=red[:, 1:2])
                    red2 = small.tile([1, 2], F32)
                    nc.vector.tensor_reduce(out=red2, in_=rowt, axis=AXX, op=OP.add)
                    e1 = small.tile([1, 1], F32)
                    nc.scalar.activation(out=e1, in_=red2[0:1, 1:2], func=AF.Exp, scale=1.0 / B, bias=half)
                    lw = small.tile([1, 1], F32)
                    nc.scalar.activation(out=lw, in_=e1, func=AF.Ln, bias=1.0)
                    lw10 = small.tile([1, 1], F32)
                    nc.scalar.mul(lw10, lw, float(LAMDA))
                    loss_sb = small.tile([1, 1], F32)
                    nc.vector.scalar_tensor_tensor(out=loss_sb, in0=red2[0:1, 0:1], scalar=1.0 / B,
                                                   in1=lw10, op0=OP.mult, op1=OP.add)
                    nc.sync.dma_start(out=loss_d[:, :], in_=loss_sb)
    nc.finalize()
    return nc


_NC_CACHE = {}


def _get_nc():
    if "nc" not in _NC_CACHE:
        _NC_CACHE["nc"] = _build()
    return _NC_CACHE["nc"]


def _make_in_maps(input, weight, class_weight, label):
    x = np.ascontiguousarray(np.asarray(input, dtype=np.float32))
    w = np.ascontiguousarray(np.asarray(weight, dtype=np.float32))
    cw = np.ascontiguousarray(np.asarray(class_weight, dtype=np.float32))
    lab = np.asarray(label).astype(np.int64)

    wpad = np.concatenate([w, np.broadcast_to(w[0:1], (CP - C, D))], axis=0)
    cwpad = np.concatenate([cw, np.full(CP - C, -1e4, dtype=np.float32)])

    xT = np.ascontiguousarray(x.T)
    wlT = np.ascontiguousarray(w[lab].T)
    cwl = np.ascontiguousarray(cw[lab])
    identb = np.tile(np.eye(128, dtype=np.float32), (1, BT))

    in_maps = []
    for c in CORES:
        sl = slice(c * CL, (c + 1) * CL)
        in_maps.append({
            "xT": xT,
            "wT": np.ascontiguousarray(wpad[sl].T),
            "wlT": wlT,
            "cw": cwpad[sl].reshape(1, CL),
            "cwl": cwl,
            "identb": identb,
        })
    return in_maps


def run(input, weight, class_weight, label, trace=False):
    in_maps = _make_in_maps(input, weight, class_weight, label)
    nc = _get_nc()
    res = run_bass_kernel_spmd(nc, in_maps, CORES, trace=trace)
    fea = np.concatenate([res.results[c]["fea"] for c in CORES], axis=1)[:, :C]
    loss = np.float32(res.results[0]["loss"][0, 0])
    return (fea, loss), res


def kernel(input, weight, class_weight, label):
    (fea, loss), _ = run(input, weight, class_weight, label)
    return fea, loss


# revision 42
# speedup vs baseline: 1.0465x; 1.0465x over previous
"""AdaptiveMarginLoss distributed Trainium2 kernel (8 NeuronCores, classifier-parallel).

Self-contained: builds + runs a Bass/Tile SPMD kernel over 8 cores.
Shards the class dimension C (padded 10000 -> 10240 = 8 * 1280).

Per core c (CL = 1280 classes):
  fea_c   = l2norm_rows(x) @ l2norm_rows(w_c).T                  [1024, 1280]
  S_c[i]  = sum_j exp(G*(fea_c[i,j] - sp[i] + B*cw_c[j]))        (+cw via K=1 matmul)
  mx_c[i] = max_j!=label masked scores (diag ~1.0 killed by relu threshold)
  AllGather(S_c - corr/8, mx_c) -> global logsumexp / max -> loss (on device)
All l2 normalization on device; rsqrt computed as exp(-0.5*ln(ssq)) so every
scalar-engine op lives in one ACT table set (no LoadActFuncSet thrash).
"""
import sys, os
sys.path.insert(0, "/opt/trn_rl_repo")
import numpy as np

import concourse.bass as bass
from concourse import bacc
import concourse.mybir as mybir
import concourse.tile as tile
from concourse.bass_utils import run_bass_kernel_spmd

B, D, C = 1024, 512, 10000
CP = 10240          # padded class count
CL = CP // 8        # classes per core
KC = D // 128       # contraction chunks (4)
BT = B // 128       # batch tiles (8)
GAMMA, BETA, LAMDA = 9.6, 0.83, 10.0
NCHUNKS = [(0, 512), (512, 512), (1024, 256)]
CORES = list(range(8))
F32 = mybir.dt.float32
F32R = mybir.dt.float32r
AF = mybir.ActivationFunctionType
OP = mybir.AluOpType


def _build(repeat=1):
    nc = bacc.Bacc()
    AXX = mybir.AxisListType.X
    xT_d = nc.declare_dram_parameter("xT", [D, B], F32, isOutput=False)
    wT_d = nc.declare_dram_parameter("wT", [D, CL], F32, isOutput=False)
    wlT_d = nc.declare_dram_parameter("wlT", [D, B], F32, isOutput=False)
    cw_d = nc.declare_dram_parameter("cw", [1, CL], F32, isOutput=False)
    cwl_d = nc.declare_dram_parameter("cwl", [B], F32, isOutput=False)

    id_d = nc.declare_dram_parameter("identb", [128, B], F32, isOutput=False)
    fea_d = nc.declare_dram_parameter("fea", [B, CL], F32, isOutput=True)
    loss_d = nc.declare_dram_parameter("loss", [1, 1], F32, isOutput=True)

    cc_in1 = nc.dram_tensor("cc_in1", [B], F32)
    cc_out1 = nc.dram_tensor("cc_out1", [8, B], F32, addr_space="Shared")
    cc_in2 = nc.dram_tensor("cc_in2", [B], F32)
    cc_out2 = nc.dram_tensor("cc_out2", [8, B], F32, addr_space="Shared")

    with tile.TileContext(nc) as tc:
        with (
            tc.tile_pool(name="big", bufs=1) as big,
            tc.tile_pool(name="work", bufs=2) as work,
            tc.tile_pool(name="small", bufs=1) as small,
            tc.tile_pool(name="mm", bufs=2, space="PSUM") as mmp,
            tc.tile_pool(name="aux", bufs=1, space="PSUM") as auxp,
        ):
            # ---------- loads: spread queues (per-engine HWDGE), chunked ----------
            xT_sb = big.tile([128, KC, B], F32)
            wT_sb = big.tile([128, KC, CL], F32)
            wlT_sb = big.tile([128, KC, B], F32)
            xT_r = xT_d.rearrange("(k p) b -> p k b", p=128)
            wT_r = wT_d.rearrange("(k p) c -> p k c", p=128)
            wlT_r = wlT_d.rearrange("(k p) b -> p k b", p=128)
            for k in range(KC):
                (nc.sync if k % 2 == 0 else nc.gpsimd).dma_start(out=wT_sb[:, k, :], in_=wT_r[:, k, :])
            for k in range(KC):
                nc.sync.dma_start(out=xT_sb[:, k, :], in_=xT_r[:, k, :])
                nc.gpsimd.dma_start(out=wlT_sb[:, k, :], in_=wlT_r[:, k, :])
            cw_row = small.tile([1, CL], F32)
            nc.gpsimd.dma_start(out=cw_row, in_=cw_d[:, :])
            cwl_pm = small.tile([128, BT], F32)
            nc.gpsimd.dma_start(out=cwl_pm, in_=cwl_d.rearrange("(t p) -> p t", p=128))

            identb = small.tile([128, B], F32)
            nc.gpsimd.dma_start(out=identb, in_=id_d[:, :])
            ones_f = small.tile([128, 128], F32)
            nc.vector.memset(ones_f, 1.0)
            ones_r = small.tile([128, 128], F32R)
            nc.vector.tensor_copy(ones_r, ones_f)
            ones1_f = small.tile([1, 128], F32)
            nc.vector.memset(ones1_f, 1.0)
            ones1_r = small.tile([1, 128], F32R)
            nc.vector.tensor_copy(ones1_r, ones1_f)
            ln8neg = small.tile([128, 1], F32)
            nc.vector.memset(ln8neg, -float(np.log(8.0)))
            relu_bias = small.tile([128, 1], F32)
            nc.vector.memset(relu_bias, -0.999e4)
            half = small.tile([1, 1], F32)
            nc.vector.memset(half, 0.5)

            for _rep in range(repeat):
                # ---------- normalize: ssq via ones-matmul; rs = exp(-0.5 ln(ssq)) ----------
                def normalize(src, n, sq_engine, mul_engine, nm):
                    dst = big.tile([128, KC, n], F32R, tag=f"norm_{nm}", bufs=1)
                    if nm == "w":
                        ssq = mmp.tile([128, n], F32, tag="mm", name=f"ssq_{nm}")
                    else:
                        ssq = auxp.tile([128, n], F32, tag="aux", name=f"ssq_{nm}")
                    for k in range(KC):
                        sq = work.tile([128, n], F32R, tag="sq", name=f"sq_{nm}_{k}")
                        sq_engine(out=sq, in0=src[:, k, :], in1=src[:, k, :])
                        for n0 in range(0, n, 512):
                            ns = min(512, n - n0)
                            nc.tensor.matmul(ssq[:, n0:n0 + ns], ones_r, sq[:, n0:n0 + ns],
                                             start=(k == 0), stop=(k == KC - 1))
                    lns = work.tile([128, n], F32, tag="rs", name=f"lns_{nm}")
                    nc.scalar.activation(out=lns, in_=ssq, func=AF.Ln)
                    rs = work.tile([128, n], F32, tag="rs", name=f"rs_{nm}")
                    nc.scalar.activation(out=rs, in_=lns, func=AF.Exp, scale=-0.5)
                    _rs_cache[nm] = rs
                    for k in range(KC):
                        mul_engine(out=dst[:, k, :], in0=src[:, k, :], in1=rs)
                    return dst
                _rs_cache = {}

                def dve_mul(out, in0, in1):
                    nc.vector.tensor_mul(out, in0, in1)

                def gps_mul(out, in0, in1):
                    nc.gpsimd.tensor_mul(out, in0, in1)

                def act_sq(out, in0, in1):
                    nc.scalar.activation(out=out, in_=in0, func=AF.Square)

                wTn = normalize(wT_sb, CL, dve_mul, dve_mul, "w")
                xTn = normalize(xT_sb, B, dve_mul, dve_mul, "x")
                # raw wl: cast to f32r; scores get 1/||wl|| via relu-scale + max post-scale
                wlT_r8 = big.tile([128, KC, B], F32R, tag="norm_wl", bufs=1)
                for k in range(KC):
                    nc.gpsimd.tensor_copy(wlT_r8[:, k, :], wlT_sb[:, k, :])
                # ssq_wl (column sums of wl^2) -> diag -> al_all = rsqrt
                ssq_wl = auxp.tile([128, B], F32, tag="aux", name="ssq_wl")
                for k in range(KC):
                    sqwl = work.tile([128, B], F32R, tag="sq", name=f"sq_wl_{k}")
                    nc.scalar.activation(out=sqwl, in_=wlT_sb[:, k, :], func=AF.Square)
                    for n0 in range(0, B, 512):
                        nc.tensor.matmul(ssq_wl[:, n0:n0 + 512], ones_r, sqwl[:, n0:n0 + 512],
                                         start=(k == 0), stop=(k == KC - 1))
                dgwl = work.tile([128, B], F32, tag="prod", name="dgwl")
                nc.vector.tensor_mul(dgwl, ssq_wl, identb)
                ssq_wl_diag = small.tile([128, BT], F32)
                nc.vector.tensor_reduce(out=ssq_wl_diag,
                                        in_=dgwl.rearrange("p (t i) -> p t i", i=128),
                                        axis=AXX, op=OP.add)
                al_ln = small.tile([128, BT], F32)
                nc.scalar.activation(out=al_ln, in_=ssq_wl_diag, func=AF.Ln)
                al_all = small.tile([128, BT], F32)
                nc.scalar.activation(out=al_all, in_=al_ln, func=AF.Exp, scale=-0.5)
                al14_all = small.tile([128, BT], F32)
                nc.vector.tensor_scalar_mul(al14_all, al_all, 1e4)
                # a_all = diag of rs_x broadcast (rs_x still live from normalize)
                dga = work.tile([128, B], F32, tag="prod", name="dga")
                nc.vector.tensor_mul(dga, _rs_cache["x"], identb)
                a_all = small.tile([128, BT], F32)
                nc.vector.tensor_reduce(out=a_all,
                                        in_=dga.rearrange("p (t i) -> p t i", i=128),
                                        axis=AXX, op=OP.add)

                # ---------- sp (raw rowdot, scaled by a*al afterwards) ----------
                sp_ps = auxp.tile([128, B], F32, tag="aux", name="sp_ps")
                for k in range(KC):
                    prod = work.tile([128, B], F32R, tag="prod")
                    nc.gpsimd.tensor_mul(prod, xT_sb[:, k, :], wlT_sb[:, k, :])
                    for n0 in range(0, B, 512):
                        nc.tensor.matmul(sp_ps[:, n0:n0 + 512], ones_r, prod[:, n0:n0 + 512],
                                         start=(k == 0), stop=(k == KC - 1))
                dgm = work.tile([128, B], F32, tag="prod", name="dgm")
                nc.vector.tensor_mul(dgm, sp_ps, identb)
                bias_all = small.tile([128, BT], F32)
                nc.vector.tensor_reduce(out=bias_all,
                                        in_=dgm.rearrange("p (t i) -> p t i", i=128),
                                        axis=AXX, op=OP.add)
                nc.vector.tensor_mul(bias_all, bias_all, a_all)
                nc.vector.tensor_mul(bias_all, bias_all, al_all)
                nc.vector.tensor_scalar_mul(bias_all, bias_all, -GAMMA)

                corr8 = small.tile([128, BT], F32)
                nc.scalar.activation(out=corr8, in_=cwl_pm, func=AF.Exp,
                                     scale=GAMMA * BETA, bias=ln8neg)

                # ---------- kexp = exp(G*B*cw) broadcast [128, CL] ----------
                kexp_row = small.tile([1, CL], F32R)
                nc.scalar.activation(out=kexp_row, in_=cw_row, func=AF.Exp, scale=GAMMA * BETA)
                kexp_ps = mmp.tile([128, CL], F32, tag="mm", name="kexp_ps")
                for n0 in range(0, CL, 512):
                    ns = min(512, CL - n0)
                    nc.tensor.matmul(kexp_ps[:, n0:n0 + ns], ones1_r, kexp_row[:, n0:n0 + ns],
                                     start=True, stop=True)
                kexp = big.tile([128, CL], F32, tag="kexp", bufs=1)
                nc.scalar.copy(kexp, kexp_ps)

                # ---------- main loop over batch tiles ----------
                HT = BT // 2
                S_h = [small.tile([128, HT], F32, name=f"S_h{h}") for h in range(2)]
                mx_h = [small.tile([128, HT], F32, name=f"mx_h{h}") for h in range(2)]
                for t in range(BT):
                    bsl = slice(t * 128, (t + 1) * 128)
                    pf = mmp.tile([128, CL], F32, tag="mm", name=f"pf{t}")
                    for k in range(KC):
                        for (n0, ns) in NCHUNKS:
                            nc.tensor.matmul(pf[:, n0:n0 + ns], xTn[:, k, bsl],
                                             wTn[:, k, n0:n0 + ns],
                                             start=(k == 0), stop=(k == KC - 1))
                    fea_sb = work.tile([128, CL], F32, tag="fea")
                    nc.scalar.copy(fea_sb, pf)
                    nc.sync.dma_start(out=fea_d[bsl, :], in_=fea_sb)
                    # scores
                    ps_ = mmp.tile([128, CL], F32, tag="mm", name=f"ps{t}")
                    for k in range(KC):
                        for (n0, ns) in NCHUNKS:
                            nc.tensor.matmul(ps_[:, n0:n0 + ns], wlT_r8[:, k, bsl],
                                             wTn[:, k, n0:n0 + ns],
                                             start=(k == 0), stop=(k == KC - 1))
                    r2 = work.tile([128, CL], F32, tag="r2")
                    nc.scalar.activation(out=r2, in_=ps_, func=AF.Relu, scale=al14_all[:, t:t + 1], bias=relu_bias)
                    msk = work.tile([128, CL], F32, tag="msk")
                    nc.vector.scalar_tensor_tensor(out=msk, in0=r2, scalar=-1e9, in1=ps_,
                                                   op0=OP.mult, op1=OP.add)
                    nc.vector.tensor_reduce(out=mx_h[t // 4][:, t % 4:t % 4 + 1], in_=msk,
                                            axis=AXX, op=OP.max)
                    E = work.tile([128, CL], F32, tag="E")
                    nc.scalar.activation(out=E, in_=pf, func=AF.Exp,
                                         bias=bias_all[:, t:t + 1], scale=GAMMA)
                    Ek = work.tile([128, CL], F32, tag="Ek")
                    nc.vector.scalar_tensor_tensor(out=Ek, in0=E, scalar=1.0, in1=kexp,
                                                   op0=OP.mult, op1=OP.mult,
                                                   accum_out=S_h[t // 4][:, t % 4:t % 4 + 1])

                # ---------- split collectives: one AllGather per half ----------
                if _rep == repeat - 1:
                    HB = HT * 128
                    red_halves = []
                    for h, (ci, co) in enumerate(((cc_in1, cc_out1), (cc_in2, cc_out2))):
                        hsl = slice(h * HT, (h + 1) * HT)
                        nc.vector.tensor_mul(mx_h[h], mx_h[h], al_all[:, hsl])
                        S_adj = small.tile([128, HT], F32, name=f"S_adj{h}")
                        nc.vector.tensor_sub(S_adj, S_h[h], corr8[:, hsl])
                        nc.sync.dma_start(out=ci[0:HB].rearrange("(t p) -> p t", p=128), in_=S_adj)
                        nc.gpsimd.dma_start(out=ci[HB:2 * HB].rearrange("(t p) -> p t", p=128), in_=mx_h[h])
                        nc.gpsimd.collective_compute(
                            "AllGather", OP.bypass,
                            ins=[ci[:]], outs=[co[:, :]],
                            replica_groups=[CORES],
                        )
                        gS = small.tile([128, HT, 8], F32, name=f"gS{h}")
                        gM = small.tile([128, HT, 8], F32, name=f"gM{h}")
                        engs = [nc.sync, nc.gpsimd]
                        for t in range(HT):
                            engs[t % 2].dma_start(out=gS[:, t, :], in_=co[:, t * 128:(t + 1) * 128].rearrange("r p -> p r"))
                            engs[(t + 1) % 2].dma_start(out=gM[:, t, :], in_=co[:, HB + t * 128:HB + (t + 1) * 128].rearrange("r p -> p r"))
                        S_tot = small.tile([128, HT], F32, name=f"S_tot{h}")
                        nc.vector.tensor_reduce(out=S_tot, in_=gS, axis=AXX, op=OP.add)
                        mx_tot = small.tile([128, HT], F32, name=f"mx_tot{h}")
                        nc.vector.tensor_reduce(out=mx_tot, in_=gM, axis=AXX, op=OP.max)
                        loss1 = small.tile([128, HT], F32, name=f"loss1{h}")
                        nc.scalar.activation(out=loss1, in_=S_tot, func=AF.Ln, bias=1.0)
                        redh = small.tile([128, 2], F32, name=f"redh{h}")
                        nc.vector.tensor_reduce(out=redh[:, 0:1], in_=loss1, axis=AXX, op=OP.add)
                        nc.vector.tensor_reduce(out=redh[:, 1:2], in_=mx_tot, axis=AXX, op=OP.add)
                        red_halves.append(redh)
                    red = small.tile([128, 2], F32)
                    nc.vector.tensor_add(red, red_halves[0], red_halves[1])
                    rowt = small.tile([1, 2, 128], F32)
                    nc.gpsimd.dma_start(out=rowt[0:1, 0, :], in_=red[:, 0:1])
                    nc.sync.dma_start(out=rowt[0:1, 1, :], in_=red[:, 1:2])
                    red2 = small.tile([1, 2], F32)
                    nc.vector.tensor_reduce(out=red2, in_=rowt, axis=AXX, op=OP.add)
                    e1 = small.tile([1, 1], F32)
                    nc.scalar.activation(out=e1, in_=red2[0:1, 1:2], func=AF.Exp, scale=1.0 / B, bias=half)
                    lw = small.tile([1, 1], F32)
                    nc.scalar.activation(out=lw, in_=e1, func=AF.Ln, bias=1.0)
                    lw10 = small.tile([1, 1], F32)
                    nc.scalar.mul(lw10, lw, float(LAMDA))
                    loss_sb = small.tile([1, 1], F32)
                    nc.vector.scalar_tensor_tensor(out=loss_sb, in0=red2[0:1, 0:1], scalar=1.0 / B,
                                                   in1=lw10, op0=OP.mult, op1=OP.add)
                    nc.sync.dma_start(out=loss_d[:, :], in_=loss_sb)
    nc.finalize()
    return nc


_NC_CACHE = {}


def _get_nc():
    if "nc" not in _NC_CACHE:
        _NC_CACHE["nc"] = _build()
    return _NC_CACHE["nc"]


def _make_in_maps(input, weight, class_weight, label):
    x = np.ascontiguousarray(np.asarray(input, dtype=np.float32))
    w = np.ascontiguousarray(np.asarray(weight, dtype=np.float32))
    cw = np.ascontiguousarray(np.asarray(class_weight, dtype=np.float32))
    lab = np.asarray(label).astype(np.int64)

    wpad = np.concatenate([w, np.broadcast_to(w[0:1], (CP - C, D))], axis=0)
    cwpad = np.concatenate([cw, np.full(CP - C, -1e4, dtype=np.float32)])

    xT = np.ascontiguousarray(x.T)
    wlT = np.ascontiguousarray(w[lab].T)
    cwl = np.ascontiguousarray(cw[lab])
    identb = np.tile(np.eye(128, dtype=np.float32), (1, BT))

    in_maps = []
    for c in CORES:
        sl = slice(c * CL, (c + 1) * CL)
        in_maps.append({
            "xT": xT,
            "wT": np.ascontiguousarray(wpad[sl].T),
            "wlT": wlT,
            "cw": cwpad[sl].reshape(1, CL),
            "cwl": cwl,
            "identb": identb,
        })
    return in_maps


def run(input, weight, class_weight, label, trace=False):
    in_maps = _make_in_maps(input, weight, class_weight, label)
    nc = _get_nc()
    res = run_bass_kernel_spmd(nc, in_maps, CORES, trace=trace)
    fea = np.concatenate([res.results[c]["fea"] for c in CORES], axis=1)[:, :C]
    loss = np.float32(res.results[0]["loss"][0, 0])
    return (fea, loss), res


def kernel(input, weight, class_weight, label):
    (fea, loss), _ = run(input, weight, class_weight, label)
    return fea, loss


# revision 50
# speedup vs baseline: 1.0580x; 1.0110x over previous
"""AdaptiveMarginLoss distributed Trainium2 kernel (8 NeuronCores, classifier-parallel).

Self-contained: builds + runs a Bass/Tile SPMD kernel over 8 cores.
Shards the class dimension C (padded 10000 -> 10240 = 8 * 1280).

Per core c (CL = 1280 classes):
  fea_c   = l2norm_rows(x) @ l2norm_rows(w_c).T                  [1024, 1280]
  S_c[i]  = sum_j exp(G*(fea_c[i,j] - sp[i] + B*cw_c[j]))        (+cw via K=1 matmul)
  mx_c[i] = max_j!=label masked scores (diag ~1.0 killed by relu threshold)
  AllGather(S_c - corr/8, mx_c) -> global logsumexp / max -> loss (on device)
All l2 normalization on device; rsqrt computed as exp(-0.5*ln(ssq)) so every
scalar-engine op lives in one ACT table set (no LoadActFuncSet thrash).
"""
import sys, os
sys.path.insert(0, "/opt/trn_rl_repo")
import numpy as np

import concourse.bass as bass
from concourse import bacc
import concourse.mybir as mybir
import concourse.tile as tile
from concourse.bass_utils import run_bass_kernel_spmd

B, D, C = 1024, 512, 10000
CP = 10240          # padded class count
CL = CP // 8        # classes per core
KC = D // 128       # contraction chunks (4)
BT = B // 128       # batch tiles (8)
GAMMA, BETA, LAMDA = 9.6, 0.83, 10.0
NCHUNKS = [(0, 512), (512, 512), (1024, 256)]
CORES = list(range(8))
F32 = mybir.dt.float32
F32R = mybir.dt.float32r
AF = mybir.ActivationFunctionType
OP = mybir.AluOpType


def _build(repeat=1):
    nc = bacc.Bacc()
    AXX = mybir.AxisListType.X
    xT_d = nc.declare_dram_parameter("xT", [D, B], F32, isOutput=False)
    wT_d = nc.declare_dram_parameter("wT", [D, CL], F32, isOutput=False)
    wlT_d = nc.declare_dram_parameter("wlT", [D, B], F32, isOutput=False)
    cw_d = nc.declare_dram_parameter("cw", [1, CL], F32, isOutput=False)
    cwl_d = nc.declare_dram_parameter("cwl", [B], F32, isOutput=False)

    id_d = nc.declare_dram_parameter("identb", [128, B], F32, isOutput=False)
    fea_d = nc.declare_dram_parameter("fea", [B, CL], F32, isOutput=True)
    loss_d = nc.declare_dram_parameter("loss", [1, 1], F32, isOutput=True)

    cc_in1 = nc.dram_tensor("cc_in1", [B], F32)
    cc_out1 = nc.dram_tensor("cc_out1", [8, B], F32, addr_space="Shared")
    cc_in2 = nc.dram_tensor("cc_in2", [B], F32)
    cc_out2 = nc.dram_tensor("cc_out2", [8, B], F32, addr_space="Shared")

    with tile.TileContext(nc) as tc:
        with (
            tc.tile_pool(name="big", bufs=1) as big,
            tc.tile_pool(name="work", bufs=2) as work,
            tc.tile_pool(name="small", bufs=1) as small,
            tc.tile_pool(name="mm", bufs=2, space="PSUM") as mmp,
            tc.tile_pool(name="aux", bufs=1, space="PSUM") as auxp,
        ):
            # ---------- loads: spread queues (per-engine HWDGE), chunked ----------
            xT_sb = big.tile([128, KC, B], F32)
            wT_sb = big.tile([128, KC, CL], F32)
            wlT_sb = big.tile([128, KC, B], F32)
            xT_r = xT_d.rearrange("(k p) b -> p k b", p=128)
            wT_r = wT_d.rearrange("(k p) c -> p k c", p=128)
            wlT_r = wlT_d.rearrange("(k p) b -> p k b", p=128)
            for k in range(KC):
                (nc.sync if k % 2 == 0 else nc.gpsimd).dma_start(out=wT_sb[:, k, :], in_=wT_r[:, k, :])
            for k in range(KC):
                nc.sync.dma_start(out=xT_sb[:, k, :], in_=xT_r[:, k, :])
                nc.gpsimd.dma_start(out=wlT_sb[:, k, :], in_=wlT_r[:, k, :])
            cw_row = small.tile([1, CL], F32)
            nc.gpsimd.dma_start(out=cw_row, in_=cw_d[:, :])
            cwl_pm = small.tile([128, BT], F32)
            nc.gpsimd.dma_start(out=cwl_pm, in_=cwl_d.rearrange("(t p) -> p t", p=128))

            identb = small.tile([128, B], F32)
            nc.gpsimd.dma_start(out=identb, in_=id_d[:, :])
            ones_f = small.tile([128, 128], F32)
            nc.vector.memset(ones_f, 1.0)
            ones_r = small.tile([128, 128], F32R)
            nc.vector.tensor_copy(ones_r, ones_f)
            ones1_f = small.tile([1, 128], F32)
            nc.vector.memset(ones1_f, 1.0)
            ones1_r = small.tile([1, 128], F32R)
            nc.vector.tensor_copy(ones1_r, ones1_f)
            ln8neg = small.tile([128, 1], F32)
            nc.vector.memset(ln8neg, -float(np.log(8.0)))
            relu_bias = small.tile([128, 1], F32)
            nc.vector.memset(relu_bias, -0.999e4)
            half = small.tile([1, 1], F32)
            nc.vector.memset(half, 0.5)

            for _rep in range(repeat):
                # ---------- normalize: ssq via ones-matmul; rs = exp(-0.5 ln(ssq)) ----------
                def normalize(src, n, sq_engine, mul_engine, nm):
                    dst = big.tile([128, KC, n], F32R, tag=f"norm_{nm}", bufs=1)
                    if nm == "w":
                        ssq = mmp.tile([128, n], F32, tag="mm", name=f"ssq_{nm}")
                    else:
                        ssq = auxp.tile([128, n], F32, tag="aux", name=f"ssq_{nm}")
                    for k in range(KC):
                        sq = work.tile([128, n], F32R, tag="sq", name=f"sq_{nm}_{k}")
                        sq_engine(out=sq, in0=src[:, k, :], in1=src[:, k, :])
                        for n0 in range(0, n, 512):
                            ns = min(512, n - n0)
                            nc.tensor.matmul(ssq[:, n0:n0 + ns], ones_r, sq[:, n0:n0 + ns],
                                             start=(k == 0), stop=(k == KC - 1))
                    lns = work.tile([128, n], F32, tag="rs", name=f"lns_{nm}")
                    nc.scalar.activation(out=lns, in_=ssq, func=AF.Ln)
                    rs = work.tile([128, n], F32, tag="rs", name=f"rs_{nm}")
                    nc.scalar.activation(out=rs, in_=lns, func=AF.Exp, scale=-0.5)
                    _rs_cache[nm] = rs
                    for k in range(KC):
                        mul_engine(out=dst[:, k, :], in0=src[:, k, :], in1=rs)
                    return dst
                _rs_cache = {}

                def dve_mul(out, in0, in1):
                    nc.vector.tensor_mul(out, in0, in1)

                def gps_mul(out, in0, in1):
                    nc.gpsimd.tensor_mul(out, in0, in1)

                def act_sq(out, in0, in1):
                    nc.scalar.activation(out=out, in_=in0, func=AF.Square)

                wTn = normalize(wT_sb, CL, dve_mul, dve_mul, "w")
                # raw x and wl: cast to f32r; 1/||.|| folded into per-partition scales
                xT_r8 = big.tile([128, KC, B], F32R, tag="norm_x", bufs=1)
                wlT_r8 = big.tile([128, KC, B], F32R, tag="norm_wl", bufs=1)
                for k in range(KC):
                    nc.scalar.copy(xT_r8[:, k, :], xT_sb[:, k, :])
                    nc.gpsimd.tensor_copy(wlT_r8[:, k, :], wlT_sb[:, k, :])
                # ssq diag chains -> merged rsqrt [128, 16] -> a_all, al_all
                sdiag = small.tile([128, 2 * BT], F32)
                for nm2, src2, sq_fn, dcol in (("x", xT_sb, dve_mul, 0), ("wl", wlT_sb, None, BT)):
                    ssq2 = auxp.tile([128, B], F32, tag="aux", name=f"ssq_{nm2}")
                    for k in range(KC):
                        sq2 = work.tile([128, B], F32R, tag="sq", name=f"sq_{nm2}_{k}")
                        if sq_fn is None:
                            nc.scalar.activation(out=sq2, in_=src2[:, k, :], func=AF.Square)
                        else:
                            sq_fn(out=sq2, in0=src2[:, k, :], in1=src2[:, k, :])
                        for n0 in range(0, B, 512):
                            nc.tensor.matmul(ssq2[:, n0:n0 + 512], ones_r, sq2[:, n0:n0 + 512],
                                             start=(k == 0), stop=(k == KC - 1))
                    dg2 = work.tile([128, B], F32, tag="prod", name=f"dg_{nm2}")
                    nc.vector.tensor_mul(dg2, ssq2, identb)
                    nc.vector.tensor_reduce(out=sdiag[:, dcol:dcol + BT],
                                            in_=dg2.rearrange("p (t i) -> p t i", i=128),
                                            axis=AXX, op=OP.add)
                sd_ln = small.tile([128, 2 * BT], F32)
                nc.scalar.activation(out=sd_ln, in_=sdiag, func=AF.Ln)
                rsq16 = small.tile([128, 2 * BT], F32)
                nc.scalar.activation(out=rsq16, in_=sd_ln, func=AF.Exp, scale=-0.5)
                a_all = rsq16[:, 0:BT]
                al_all = rsq16[:, BT:2 * BT]
                ga_all = small.tile([128, BT], F32)
                nc.vector.tensor_scalar_mul(ga_all, a_all, GAMMA)
                al14_all = small.tile([128, BT], F32)
                nc.vector.tensor_scalar_mul(al14_all, al_all, 1e4)

                # ---------- sp (raw rowdot, scaled by a*al afterwards) ----------
                sp_ps = auxp.tile([128, B], F32, tag="aux", name="sp_ps")
                for k in range(KC):
                    prod = work.tile([128, B], F32R, tag="prod")
                    nc.gpsimd.tensor_mul(prod, xT_sb[:, k, :], wlT_sb[:, k, :])
                    for n0 in range(0, B, 512):
                        nc.tensor.matmul(sp_ps[:, n0:n0 + 512], ones_r, prod[:, n0:n0 + 512],
                                         start=(k == 0), stop=(k == KC - 1))
                dgm = work.tile([128, B], F32, tag="prod", name="dgm")
                nc.vector.tensor_mul(dgm, sp_ps, identb)
                bias_all = small.tile([128, BT], F32)
                nc.vector.tensor_reduce(out=bias_all,
                                        in_=dgm.rearrange("p (t i) -> p t i", i=128),
                                        axis=AXX, op=OP.add)
                nc.vector.tensor_mul(bias_all, bias_all, a_all)
                nc.vector.tensor_mul(bias_all, bias_all, al_all)
                nc.vector.tensor_scalar_mul(bias_all, bias_all, -GAMMA)

                corr8 = small.tile([128, BT], F32)
                nc.scalar.activation(out=corr8, in_=cwl_pm, func=AF.Exp,
                                     scale=GAMMA * BETA, bias=ln8neg)

                # ---------- kexp = exp(G*B*cw) broadcast [128, CL] ----------
                kexp_row = small.tile([1, CL], F32R)
                nc.scalar.activation(out=kexp_row, in_=cw_row, func=AF.Exp, scale=GAMMA * BETA)
                kexp_ps = mmp.tile([128, CL], F32, tag="mm", name="kexp_ps")
                for n0 in range(0, CL, 512):
                    ns = min(512, CL - n0)
                    nc.tensor.matmul(kexp_ps[:, n0:n0 + ns], ones1_r, kexp_row[:, n0:n0 + ns],
                                     start=True, stop=True)
                kexp = big.tile([128, CL], mybir.dt.bfloat16, tag="kexp", bufs=1)
                nc.scalar.copy(kexp, kexp_ps)

                # ---------- main loop over batch tiles ----------
                HT = BT // 2
                S_h = [small.tile([128, HT], F32, name=f"S_h{h}") for h in range(2)]
                mx_h = [small.tile([128, HT], F32, name=f"mx_h{h}") for h in range(2)]
                for t in range(BT):
                    bsl = slice(t * 128, (t + 1) * 128)
                    pf = mmp.tile([128, CL], F32, tag="mm", name=f"pf{t}")
                    for k in range(KC):
                        for (n0, ns) in NCHUNKS:
                            nc.tensor.matmul(pf[:, n0:n0 + ns], xT_r8[:, k, bsl],
                                             wTn[:, k, n0:n0 + ns],
                                             start=(k == 0), stop=(k == KC - 1))
                    fea_sb = work.tile([128, CL], F32, tag="fea")
                    nc.scalar.activation(out=fea_sb, in_=pf, func=AF.Copy, scale=a_all[:, t:t + 1])
                    nc.sync.dma_start(out=fea_d[bsl, :], in_=fea_sb)
                    # scores
                    ps_ = mmp.tile([128, CL], F32, tag="mm", name=f"ps{t}")
                    for k in range(KC):
                        for (n0, ns) in NCHUNKS:
                            nc.tensor.matmul(ps_[:, n0:n0 + ns], wlT_r8[:, k, bsl],
                                             wTn[:, k, n0:n0 + ns],
                                             start=(k == 0), stop=(k == KC - 1))
                    r2 = work.tile([128, CL], F32, tag="r2", bufs=2)
                    nc.scalar.activation(out=r2, in_=ps_, func=AF.Relu, scale=al14_all[:, t:t + 1], bias=relu_bias)
                    msk = work.tile([128, CL], mybir.dt.bfloat16, tag="msk", bufs=3)
                    nc.vector.scalar_tensor_tensor(out=msk, in0=r2, scalar=-1e9, in1=ps_,
                                                   op0=OP.mult, op1=OP.add)
                    nc.vector.tensor_reduce(out=mx_h[t // 4][:, t % 4:t % 4 + 1], in_=msk,
                                            axis=AXX, op=OP.max)
                    E = work.tile([128, CL], mybir.dt.bfloat16, tag="E", bufs=3)
                    nc.scalar.activation(out=E, in_=pf, func=AF.Exp,
                                         bias=bias_all[:, t:t + 1], scale=ga_all[:, t:t + 1])
                    Ek = work.tile([128, CL], mybir.dt.bfloat16, tag="Ek", bufs=2)
                    nc.vector.scalar_tensor_tensor(out=Ek, in0=E, scalar=1.0, in1=kexp,
                                                   op0=OP.mult, op1=OP.mult,
                                                   accum_out=S_h[t // 4][:, t % 4:t % 4 + 1])

                # ---------- split collectives: one AllGather per half ----------
                if _rep == repeat - 1:
                    HB = HT * 128
                    red_halves = []
                    for h, (ci, co) in enumerate(((cc_in1, cc_out1), (cc_in2, cc_out2))):
                        hsl = slice(h * HT, (h + 1) * HT)
                        nc.vector.tensor_mul(mx_h[h], mx_h[h], al_all[:, hsl])
                        S_adj = small.tile([128, HT], F32, name=f"S_adj{h}")
                        nc.vector.tensor_sub(S_adj, S_h[h], corr8[:, hsl])
                        # interleaved pack: q = 2t+kind so the gathered buffer is one
                        # mergeable 2-D DMA: dram idx = (r*8 + 2t + k)*128 + p
                        ci_q = ci.rearrange("(q p) -> p q", p=128)
                        nc.sync.dma_start(out=ci_q[:, 0::2], in_=S_adj)
                        nc.gpsimd.dma_start(out=ci_q[:, 1::2], in_=mx_h[h])
                        nc.gpsimd.collective_compute(
                            "AllGather", OP.bypass,
                            ins=[ci[:]], outs=[co[:, :]],
                            replica_groups=[CORES],
                        )
                        g = small.tile([128, 8 * HT * 2], F32, name=f"g{h}")
                        nc.sync.dma_start(out=g, in_=co.rearrange("r (q p) -> p (r q)", p=128))
                        gv = g.rearrange("p (r t k) -> p t k r", r=8, t=HT)
                        addred = small.tile([128, HT, 2], F32, name=f"addred{h}")
                        nc.vector.tensor_reduce(out=addred, in_=gv, axis=AXX, op=OP.add)
                        maxred = small.tile([128, HT, 2], F32, name=f"maxred{h}")
                        nc.vector.tensor_reduce(out=maxred, in_=gv, axis=AXX, op=OP.max)
                        S_tot = addred[:, :, 0]
                        mx_tot = maxred[:, :, 1]
                        loss1 = small.tile([128, HT], F32, name=f"loss1{h}")
                        nc.scalar.activation(out=loss1, in_=S_tot, func=AF.Ln, bias=1.0)
                        redh = small.tile([128, 2], F32, name=f"redh{h}")
                        nc.vector.tensor_reduce(out=redh[:, 0:1], in_=loss1, axis=AXX, op=OP.add)
                        nc.vector.tensor_reduce(out=redh[:, 1:2], in_=mx_tot, axis=AXX, op=OP.add)
                        red_halves.append(redh)
                    red_r = small.tile([128, 2], F32R)
                    nc.vector.tensor_add(red_r, red_halves[0], red_halves[1])
                    pscal = mmp.tile([1, 2], F32, tag="mm", name="pscal")
                    nc.tensor.matmul(pscal, ones_r[:, 0:1], red_r, start=True, stop=True)
                    red2 = small.tile([1, 2], F32)
                    nc.scalar.copy(red2, pscal)
                    e1 = small.tile([1, 1], F32)
                    nc.scalar.activation(out=e1, in_=red2[0:1, 1:2], func=AF.Exp, scale=1.0 / B, bias=half)
                    lw = small.tile([1, 1], F32)
                    nc.scalar.activation(out=lw, in_=e1, func=AF.Ln, bias=1.0)
                    lw10 = small.tile([1, 1], F32)
                    nc.scalar.mul(lw10, lw, float(LAMDA))
                    loss_sb = small.tile([1, 1], F32)
                    nc.vector.scalar_tensor_tensor(out=loss_sb, in0=red2[0:1, 0:1], scalar=1.0 / B,
                                                   in1=lw10, op0=OP.mult, op1=OP.add)
                    nc.sync.dma_start(out=loss_d[:, :], in_=loss_sb)
    nc.finalize()
    return nc


_NC_CACHE = {}


def _get_nc():
    if "nc" not in _NC_CACHE:
        _NC_CACHE["nc"] = _build()
    return _NC_CACHE["nc"]


def _make_in_maps(input, weight, class_weight, label):
    x = np.ascontiguousarray(np.asarray(input, dtype=np.float32))
    w = np.ascontiguousarray(np.asarray(weight, dtype=np.float32))
    cw = np.ascontiguousarray(np.asarray(class_weight, dtype=np.float32))
    lab = np.asarray(label).astype(np.int64)

    wpad = np.concatenate([w, np.broadcast_to(w[0:1], (CP - C, D))], axis=0)
    cwpad = np.concatenate([cw, np.full(CP - C, -1e4, dtype=np.float32)])

    xT = np.ascontiguousarray(x.T)
    wlT = np.ascontiguousarray(w[lab].T)
    cwl = np.ascontiguousarray(cw[lab])
    identb = np.tile(np.eye(128, dtype=np.float32), (1, BT))

    in_maps = []
    for c in CORES:
        sl = slice(c * CL, (c + 1) * CL)
        in_maps.append({
            "xT": xT,
            "wT": np.ascontiguousarray(wpad[sl].T),
            "wlT": wlT,
            "cw": cwpad[sl].reshape(1, CL),
            "cwl": cwl,
            "identb": identb,
        })
    return in_maps


def run(input, weight, class_weight, label, trace=False):
    in_maps = _make_in_maps(input, weight, class_weight, label)
    nc = _get_nc()
    res = run_bass_kernel_spmd(nc, in_maps, CORES, trace=trace)
    fea = np.concatenate([res.results[c]["fea"] for c in CORES], axis=1)[:, :C]
    loss = np.float32(res.results[0]["loss"][0, 0])
    return (fea, loss), res


def kernel(input, weight, class_weight, label):
    (fea, loss), _ = run(input, weight, class_weight, label)
    return fea, loss


# revision 56
# speedup vs baseline: 1.0942x; 1.0342x over previous
"""AdaptiveMarginLoss distributed Trainium2 kernel (8 NeuronCores, classifier-parallel).

Self-contained: builds + runs a Bass/Tile SPMD kernel over 8 cores.
Shards the class dimension C (padded 10000 -> 10240 = 8 * 1280).

Per core c (CL = 1280 classes):
  fea_c   = l2norm_rows(x) @ l2norm_rows(w_c).T                  [1024, 1280]
  S_c[i]  = sum_j exp(G*(fea_c[i,j] - sp[i] + B*cw_c[j]))        (+cw via K=1 matmul)
  mx_c[i] = max_j!=label masked scores (diag ~1.0 killed by relu threshold)
  AllGather(S_c - corr/8, mx_c) -> global logsumexp / max -> loss (on device)
All l2 normalization on device; rsqrt computed as exp(-0.5*ln(ssq)) so every
scalar-engine op lives in one ACT table set (no LoadActFuncSet thrash).
"""
import sys, os
sys.path.insert(0, "/opt/trn_rl_repo")
import numpy as np

import concourse.bass as bass
from concourse import bacc
import concourse.mybir as mybir
import concourse.tile as tile
from concourse.bass_utils import run_bass_kernel_spmd

B, D, C = 1024, 512, 10000
CP = 10240          # padded class count
CL = CP // 8        # classes per core
KC = D // 128       # contraction chunks (4)
BT = B // 128       # batch tiles (8)
GAMMA, BETA, LAMDA = 9.6, 0.83, 10.0
NCHUNKS = [(0, 512), (512, 512), (1024, 256)]
CORES = list(range(8))
F32 = mybir.dt.float32
F32R = mybir.dt.float32r
AF = mybir.ActivationFunctionType
OP = mybir.AluOpType


def _build(repeat=1):
    nc = bacc.Bacc()
    AXX = mybir.AxisListType.X
    xT_d = nc.declare_dram_parameter("xT", [D, B], F32, isOutput=False)
    wT_d = nc.declare_dram_parameter("wT", [D, CL], F32, isOutput=False)
    wlT_d = nc.declare_dram_parameter("wlT", [D, B], F32, isOutput=False)
    cw_d = nc.declare_dram_parameter("cw", [1, CL], F32, isOutput=False)
    cwl_d = nc.declare_dram_parameter("cwl", [B], F32, isOutput=False)

    id_d = nc.declare_dram_parameter("identb", [128, B], F32, isOutput=False)
    fea_d = nc.declare_dram_parameter("fea", [B, CL], F32, isOutput=True)
    loss_d = nc.declare_dram_parameter("loss", [1, 1], F32, isOutput=True)

    cc_in1 = nc.dram_tensor("cc_in1", [B], F32)
    cc_out1 = nc.dram_tensor("cc_out1", [8, B], F32, addr_space="Shared")
    cc_in2 = nc.dram_tensor("cc_in2", [B], F32)
    cc_out2 = nc.dram_tensor("cc_out2", [8, B], F32, addr_space="Shared")

    with tile.TileContext(nc) as tc:
        with (
            tc.tile_pool(name="big", bufs=1) as big,
            tc.tile_pool(name="work", bufs=2) as work,
            tc.tile_pool(name="small", bufs=1) as small,
            tc.tile_pool(name="mm", bufs=2, space="PSUM") as mmp,
            tc.tile_pool(name="aux", bufs=1, space="PSUM") as auxp,
        ):
            # ---------- loads: spread queues (per-engine HWDGE), chunked ----------
            xT_sb = big.tile([128, KC, B], F32)
            wT_sb = big.tile([128, KC, CL], F32)
            wlT_sb = big.tile([128, KC, B], F32)
            xT_r = xT_d.rearrange("(k p) b -> p k b", p=128)
            wT_r = wT_d.rearrange("(k p) c -> p k c", p=128)
            wlT_r = wlT_d.rearrange("(k p) b -> p k b", p=128)
            for k in range(KC):
                (nc.sync if k % 2 == 0 else nc.gpsimd).dma_start(out=wT_sb[:, k, :], in_=wT_r[:, k, :])
            for k in range(KC):
                nc.sync.dma_start(out=xT_sb[:, k, :], in_=xT_r[:, k, :])
                nc.gpsimd.dma_start(out=wlT_sb[:, k, :], in_=wlT_r[:, k, :])
            cw_row = small.tile([1, CL], F32)
            nc.gpsimd.dma_start(out=cw_row, in_=cw_d[:, :])
            cwl_pm = small.tile([128, BT], F32)
            nc.gpsimd.dma_start(out=cwl_pm, in_=cwl_d.rearrange("(t p) -> p t", p=128))

            identb = small.tile([128, B], F32)
            nc.gpsimd.dma_start(out=identb, in_=id_d[:, :])
            ones_f = small.tile([128, 128], F32)
            nc.vector.memset(ones_f, 1.0)
            ones_r = small.tile([128, 128], F32R)
            nc.vector.tensor_copy(ones_r, ones_f)
            ones1_f = small.tile([1, 128], F32)
            nc.vector.memset(ones1_f, 1.0)
            ones1_r = small.tile([1, 128], F32R)
            nc.vector.tensor_copy(ones1_r, ones1_f)
            ln8neg = small.tile([128, 1], F32)
            nc.vector.memset(ln8neg, -float(np.log(8.0)))
            relu_bias = small.tile([128, 1], F32)
            nc.vector.memset(relu_bias, -0.999e4)
            half = small.tile([1, 1], F32)
            nc.vector.memset(half, 0.5)

            for _rep in range(repeat):
                # ---------- normalize: ssq via ones-matmul; rs = exp(-0.5 ln(ssq)) ----------
                def normalize(src, n, sq_engine, mul_engine, nm):
                    dst = big.tile([128, KC, n], F32R, tag=f"norm_{nm}", bufs=1)
                    if nm == "w":
                        ssq = mmp.tile([128, n], F32, tag="mm", name=f"ssq_{nm}")
                    else:
                        ssq = auxp.tile([128, n], F32, tag="aux", name=f"ssq_{nm}")
                    for k in range(KC):
                        sq = work.tile([128, n], F32R, tag="sq", name=f"sq_{nm}_{k}")
                        sq_engine(out=sq, in0=src[:, k, :], in1=src[:, k, :])
                        for n0 in range(0, n, 512):
                            ns = min(512, n - n0)
                            nc.tensor.matmul(ssq[:, n0:n0 + ns], ones_r, sq[:, n0:n0 + ns],
                                             start=(k == 0), stop=(k == KC - 1))
                    lns = work.tile([128, n], F32, tag="rs", name=f"lns_{nm}")
                    nc.scalar.activation(out=lns, in_=ssq, func=AF.Ln)
                    rs = work.tile([128, n], F32, tag="rs", name=f"rs_{nm}")
                    nc.scalar.activation(out=rs, in_=lns, func=AF.Exp, scale=-0.5)
                    _rs_cache[nm] = rs
                    for k in range(KC):
                        eng = gps_mul if k % 2 == 0 else mul_engine
                        eng(out=dst[:, k, :], in0=src[:, k, :], in1=rs)
                    return dst
                _rs_cache = {}

                def dve_mul(out, in0, in1):
                    nc.vector.tensor_mul(out, in0, in1)

                def gps_mul(out, in0, in1):
                    nc.gpsimd.tensor_mul(out, in0, in1)

                def act_sq(out, in0, in1):
                    nc.scalar.activation(out=out, in_=in0, func=AF.Square)

                wTn = normalize(wT_sb, CL, dve_mul, dve_mul, "w")
                # raw x and wl: cast to f32r; 1/||.|| folded into per-partition scales
                xT_r8 = big.tile([128, KC, B], F32R, tag="norm_x", bufs=1)
                wlT_r8 = big.tile([128, KC, B], F32R, tag="norm_wl", bufs=1)
                for k in range(KC):
                    nc.scalar.copy(xT_r8[:, k, :], xT_sb[:, k, :])
                    nc.gpsimd.tensor_copy(wlT_r8[:, k, :], wlT_sb[:, k, :])
                # ssq diag chains -> merged rsqrt [128, 16] -> a_all, al_all
                sdiag = small.tile([128, 2 * BT], F32)
                for nm2, src2, sq_fn, dcol in (("x", xT_sb, dve_mul, 0), ("wl", wlT_sb, None, BT)):
                    ssq2 = auxp.tile([128, B], F32, tag="aux", name=f"ssq_{nm2}")
                    for k in range(KC):
                        sq2 = work.tile([128, B], F32R, tag="sq", name=f"sq_{nm2}_{k}")
                        if sq_fn is None:
                            nc.scalar.activation(out=sq2, in_=src2[:, k, :], func=AF.Square)
                        else:
                            sq_fn(out=sq2, in0=src2[:, k, :], in1=src2[:, k, :])
                        for n0 in range(0, B, 512):
                            nc.tensor.matmul(ssq2[:, n0:n0 + 512], ones_r, sq2[:, n0:n0 + 512],
                                             start=(k == 0), stop=(k == KC - 1))
                    dg2 = work.tile([128, B], F32, tag="prod", name=f"dg_{nm2}")
                    nc.vector.tensor_mul(dg2, ssq2, identb)
                    nc.vector.tensor_reduce(out=sdiag[:, dcol:dcol + BT],
                                            in_=dg2.rearrange("p (t i) -> p t i", i=128),
                                            axis=AXX, op=OP.add)
                sd_ln = small.tile([128, 2 * BT], F32)
                nc.scalar.activation(out=sd_ln, in_=sdiag, func=AF.Ln)
                rsq16 = small.tile([128, 2 * BT], F32)
                nc.scalar.activation(out=rsq16, in_=sd_ln, func=AF.Exp, scale=-0.5)
                a_all = rsq16[:, 0:BT]
                al_all = rsq16[:, BT:2 * BT]
                ga_all = small.tile([128, BT], F32)
                nc.vector.tensor_scalar_mul(ga_all, a_all, GAMMA)
                al14_all = small.tile([128, BT], F32)
                nc.vector.tensor_scalar_mul(al14_all, al_all, 1e4)

                # ---------- sp (raw rowdot, scaled by a*al afterwards) ----------
                sp_ps = auxp.tile([128, B], F32, tag="aux", name="sp_ps")
                for k in range(KC):
                    prod = work.tile([128, B], F32R, tag="prod")
                    nc.gpsimd.tensor_mul(prod, xT_sb[:, k, :], wlT_sb[:, k, :])
                    for n0 in range(0, B, 512):
                        nc.tensor.matmul(sp_ps[:, n0:n0 + 512], ones_r, prod[:, n0:n0 + 512],
                                         start=(k == 0), stop=(k == KC - 1))
                dgm = work.tile([128, B], F32, tag="prod", name="dgm")
                nc.vector.tensor_mul(dgm, sp_ps, identb)
                bias_all = small.tile([128, BT], F32)
                nc.vector.tensor_reduce(out=bias_all,
                                        in_=dgm.rearrange("p (t i) -> p t i", i=128),
                                        axis=AXX, op=OP.add)
                nc.vector.tensor_mul(bias_all, bias_all, a_all)
                nc.vector.tensor_mul(bias_all, bias_all, al_all)
                nc.vector.tensor_scalar_mul(bias_all, bias_all, -GAMMA)

                corr8 = small.tile([128, BT], F32)
                nc.scalar.activation(out=corr8, in_=cwl_pm, func=AF.Exp,
                                     scale=GAMMA * BETA, bias=ln8neg)

                # ---------- kexp = exp(G*B*cw) broadcast [128, CL] ----------
                kexp_row = small.tile([1, CL], F32R)
                nc.scalar.activation(out=kexp_row, in_=cw_row, func=AF.Exp, scale=GAMMA * BETA)
                kexp_ps = mmp.tile([128, CL], F32, tag="mm", name="kexp_ps")
                for n0 in range(0, CL, 512):
                    ns = min(512, CL - n0)
                    nc.tensor.matmul(kexp_ps[:, n0:n0 + ns], ones1_r, kexp_row[:, n0:n0 + ns],
                                     start=True, stop=True)
                kexp = big.tile([128, CL], mybir.dt.bfloat16, tag="kexp", bufs=1)
                nc.scalar.copy(kexp, kexp_ps)

                # ---------- main loop over batch tiles ----------
                HT = BT // 2
                S_h = [small.tile([128, HT], F32, name=f"S_h{h}") for h in range(2)]
                mx_h = [small.tile([128, HT], F32, name=f"mx_h{h}") for h in range(2)]
                for t in range(BT):
                    bsl = slice(t * 128, (t + 1) * 128)
                    pf = mmp.tile([128, CL], F32, tag="mm", name=f"pf{t}")
                    for k in range(KC):
                        for (n0, ns) in NCHUNKS:
                            nc.tensor.matmul(pf[:, n0:n0 + ns], xT_r8[:, k, bsl],
                                             wTn[:, k, n0:n0 + ns],
                                             start=(k == 0), stop=(k == KC - 1))
                    fea_sb = work.tile([128, CL], F32, tag="fea")
                    nc.scalar.activation(out=fea_sb, in_=pf, func=AF.Copy, scale=a_all[:, t:t + 1])
                    nc.sync.dma_start(out=fea_d[bsl, :], in_=fea_sb)
                    # scores
                    ps_ = mmp.tile([128, CL], F32, tag="mm", name=f"ps{t}")
                    for k in range(KC):
                        for (n0, ns) in NCHUNKS:
                            nc.tensor.matmul(ps_[:, n0:n0 + ns], wlT_r8[:, k, bsl],
                                             wTn[:, k, n0:n0 + ns],
                                             start=(k == 0), stop=(k == KC - 1))
                    r2 = work.tile([128, CL], F32, tag="r2", bufs=2)
                    nc.scalar.activation(out=r2, in_=ps_, func=AF.Relu, scale=al14_all[:, t:t + 1], bias=relu_bias)
                    msk = work.tile([128, CL], mybir.dt.bfloat16, tag="msk", bufs=3)
                    nc.vector.scalar_tensor_tensor(out=msk, in0=r2, scalar=-1e9, in1=ps_,
                                                   op0=OP.mult, op1=OP.add)
                    nc.vector.tensor_reduce(out=mx_h[t // 4][:, t % 4:t % 4 + 1], in_=msk,
                                            axis=AXX, op=OP.max)
                    E = work.tile([128, CL], mybir.dt.bfloat16, tag="E", bufs=3)
                    nc.scalar.activation(out=E, in_=pf, func=AF.Exp,
                                         bias=bias_all[:, t:t + 1], scale=ga_all[:, t:t + 1])
                    Ek = work.tile([128, CL], mybir.dt.bfloat16, tag="Ek", bufs=2)
                    nc.vector.scalar_tensor_tensor(out=Ek, in0=E, scalar=1.0, in1=kexp,
                                                   op0=OP.mult, op1=OP.mult,
                                                   accum_out=S_h[t // 4][:, t % 4:t % 4 + 1])

                # ---------- split collectives: one AllGather per half ----------
                if _rep == repeat - 1:
                    HB = HT * 128
                    red_halves = []
                    for h, (ci, co) in enumerate(((cc_in1, cc_out1), (cc_in2, cc_out2))):
                        hsl = slice(h * HT, (h + 1) * HT)
                        nc.vector.tensor_mul(mx_h[h], mx_h[h], al_all[:, hsl])
                        S_adj = small.tile([128, HT], F32, name=f"S_adj{h}")
                        nc.vector.tensor_sub(S_adj, S_h[h], corr8[:, hsl])
                        # interleaved pack: q = 2t+kind so the gathered buffer is one
                        # mergeable 2-D DMA: dram idx = (r*8 + 2t + k)*128 + p
                        ci_q = ci.rearrange("(q p) -> p q", p=128)
                        nc.sync.dma_start(out=ci_q[:, 0::2], in_=S_adj)
                        nc.gpsimd.dma_start(out=ci_q[:, 1::2], in_=mx_h[h])
                        nc.gpsimd.collective_compute(
                            "AllGather", OP.bypass,
                            ins=[ci[:]], outs=[co[:, :]],
                            replica_groups=[CORES],
                        )
                        g = small.tile([128, 8 * HT * 2], F32, name=f"g{h}")
                        nc.sync.dma_start(out=g, in_=co.rearrange("r (q p) -> p (r q)", p=128))
                        gv = g.rearrange("p (r t k) -> p t k r", r=8, t=HT)
                        addred = small.tile([128, HT, 2], F32, name=f"addred{h}")
                        nc.vector.tensor_reduce(out=addred, in_=gv, axis=AXX, op=OP.add)
                        maxred = small.tile([128, HT, 2], F32, name=f"maxred{h}")
                        nc.vector.tensor_reduce(out=maxred, in_=gv, axis=AXX, op=OP.max)
                        S_tot = addred[:, :, 0]
                        mx_tot = maxred[:, :, 1]
                        loss1 = small.tile([128, HT], F32, name=f"loss1{h}")
                        nc.scalar.activation(out=loss1, in_=S_tot, func=AF.Ln, bias=1.0)
                        redh = small.tile([128, 2], F32, name=f"redh{h}")
                        nc.vector.tensor_reduce(out=redh[:, 0:1], in_=loss1, axis=AXX, op=OP.add)
                        nc.vector.tensor_reduce(out=redh[:, 1:2], in_=mx_tot, axis=AXX, op=OP.add)
                        red_halves.append(redh)
                    red_r = small.tile([128, 2], F32R)
                    nc.vector.tensor_add(red_r, red_halves[0], red_halves[1])
                    pscal = mmp.tile([1, 2], F32, tag="mm", name="pscal")
                    nc.tensor.matmul(pscal, ones_r[:, 0:1], red_r, start=True, stop=True)
                    red2 = small.tile([1, 2], F32)
                    nc.scalar.copy(red2, pscal)
                    e1 = small.tile([1, 1], F32)
                    nc.scalar.activation(out=e1, in_=red2[0:1, 1:2], func=AF.Exp, scale=1.0 / B, bias=half)
                    lw = small.tile([1, 1], F32)
                    nc.scalar.activation(out=lw, in_=e1, func=AF.Ln, bias=1.0)
                    lw10 = small.tile([1, 1], F32)
                    nc.scalar.mul(lw10, lw, float(LAMDA))
                    loss_sb = small.tile([1, 1], F32)
                    nc.vector.scalar_tensor_tensor(out=loss_sb, in0=red2[0:1, 0:1], scalar=1.0 / B,
                                                   in1=lw10, op0=OP.mult, op1=OP.add)
                    nc.sync.dma_start(out=loss_d[:, :], in_=loss_sb)
    nc.finalize()
    return nc


_NC_CACHE = {}


def _get_nc():
    if "nc" not in _NC_CACHE:
        _NC_CACHE["nc"] = _build()
    return _NC_CACHE["nc"]


def _make_in_maps(input, weight, class_weight, label):
    x = np.ascontiguousarray(np.asarray(input, dtype=np.float32))
    w = np.ascontiguousarray(np.asarray(weight, dtype=np.float32))
    cw = np.ascontiguousarray(np.asarray(class_weight, dtype=np.float32))
    lab = np.asarray(label).astype(np.int64)

    wpad = np.concatenate([w, np.broadcast_to(w[0:1], (CP - C, D))], axis=0)
    cwpad = np.concatenate([cw, np.full(CP - C, -1e4, dtype=np.float32)])

    xT = np.ascontiguousarray(x.T)
    wlT = np.ascontiguousarray(w[lab].T)
    cwl = np.ascontiguousarray(cw[lab])
    identb = np.tile(np.eye(128, dtype=np.float32), (1, BT))

    in_maps = []
    for c in CORES:
        sl = slice(c * CL, (c + 1) * CL)
        in_maps.append({
            "xT": xT,
            "wT": np.ascontiguousarray(wpad[sl].T),
            "wlT": wlT,
            "cw": cwpad[sl].reshape(1, CL),
            "cwl": cwl,
            "identb": identb,
        })
    return in_maps


def run(input, weight, class_weight, label, trace=False):
    in_maps = _make_in_maps(input, weight, class_weight, label)
    nc = _get_nc()
    res = run_bass_kernel_spmd(nc, in_maps, CORES, trace=trace)
    fea = np.concatenate([res.results[c]["fea"] for c in CORES], axis=1)[:, :C]
    loss = np.float32(res.results[0]["loss"][0, 0])
    return (fea, loss), res


def kernel(input, weight, class_weight, label):
    (fea, loss), _ = run(input, weight, class_weight, label)
    return fea, loss


# revision 59
# speedup vs baseline: 1.1104x; 1.0149x over previous
"""AdaptiveMarginLoss distributed Trainium2 kernel (8 NeuronCores, classifier-parallel).

Self-contained: builds + runs a Bass/Tile SPMD kernel over 8 cores.
Shards the class dimension C (padded 10000 -> 10240 = 8 * 1280).

Per core c (CL = 1280 classes):
  fea_c   = l2norm_rows(x) @ l2norm_rows(w_c).T                  [1024, 1280]
  S_c[i]  = sum_j exp(G*(fea_c[i,j] - sp[i] + B*cw_c[j]))        (+cw via K=1 matmul)
  mx_c[i] = max_j!=label masked scores (diag ~1.0 killed by relu threshold)
  AllGather(S_c - corr/8, mx_c) -> global logsumexp / max -> loss (on device)
All l2 normalization on device; rsqrt computed as exp(-0.5*ln(ssq)) so every
scalar-engine op lives in one ACT table set (no LoadActFuncSet thrash).
"""
import sys, os
sys.path.insert(0, "/opt/trn_rl_repo")
import numpy as np

import concourse.bass as bass
from concourse import bacc
import concourse.mybir as mybir
import concourse.tile as tile
from concourse.bass_utils import run_bass_kernel_spmd

B, D, C = 1024, 512, 10000
CP = 10240          # padded class count
CL = CP // 8        # classes per core
KC = D // 128       # contraction chunks (4)
BT = B // 128       # batch tiles (8)
GAMMA, BETA, LAMDA = 9.6, 0.83, 10.0
NCHUNKS = [(0, 512), (512, 512), (1024, 256)]
CORES = list(range(8))
F32 = mybir.dt.float32
F32R = mybir.dt.float32r
AF = mybir.ActivationFunctionType
OP = mybir.AluOpType


def _build(repeat=1):
    nc = bacc.Bacc()
    AXX = mybir.AxisListType.X
    xT_d = nc.declare_dram_parameter("xT", [D, B], F32, isOutput=False)
    wT_d = nc.declare_dram_parameter("wT", [D, CL], F32, isOutput=False)
    wlT_d = nc.declare_dram_parameter("wlT", [D, B], F32, isOutput=False)
    cw_d = nc.declare_dram_parameter("cw", [1, CL], F32, isOutput=False)
    cwl_d = nc.declare_dram_parameter("cwl", [B], F32, isOutput=False)

    id_d = nc.declare_dram_parameter("identb", [128, B], F32, isOutput=False)
    fea_d = nc.declare_dram_parameter("fea", [B, CL], F32, isOutput=True)
    loss_d = nc.declare_dram_parameter("loss", [1, 1], F32, isOutput=True)

    cc_in1 = nc.dram_tensor("cc_in1", [B], F32)
    cc_out1 = nc.dram_tensor("cc_out1", [8, B], F32, addr_space="Shared")
    cc_in2 = nc.dram_tensor("cc_in2", [B], F32)
    cc_out2 = nc.dram_tensor("cc_out2", [8, B], F32, addr_space="Shared")

    with tile.TileContext(nc) as tc:
        with (
            tc.tile_pool(name="big", bufs=1) as big,
            tc.tile_pool(name="work", bufs=2) as work,
            tc.tile_pool(name="small", bufs=1) as small,
            tc.tile_pool(name="mm", bufs=2, space="PSUM") as mmp,
            tc.tile_pool(name="aux", bufs=1, space="PSUM") as auxp,
        ):
            # ---------- loads: spread queues (per-engine HWDGE), chunked ----------
            xT_sb = big.tile([128, KC, B], F32)
            wT_sb = big.tile([128, KC, CL], F32)
            wlT_sb = big.tile([128, KC, B], F32)
            xT_r = xT_d.rearrange("(k p) b -> p k b", p=128)
            wT_r = wT_d.rearrange("(k p) c -> p k c", p=128)
            wlT_r = wlT_d.rearrange("(k p) b -> p k b", p=128)
            for k in range(KC):
                (nc.sync if k % 2 == 0 else nc.gpsimd).dma_start(out=wT_sb[:, k, :], in_=wT_r[:, k, :])
            for k in range(KC):
                nc.sync.dma_start(out=xT_sb[:, k, :], in_=xT_r[:, k, :])
                nc.gpsimd.dma_start(out=wlT_sb[:, k, :], in_=wlT_r[:, k, :])
            cw_row = small.tile([1, CL], F32)
            nc.gpsimd.dma_start(out=cw_row, in_=cw_d[:, :])
            cwl_pm = small.tile([128, BT], F32)
            nc.gpsimd.dma_start(out=cwl_pm, in_=cwl_d.rearrange("(t p) -> p t", p=128))

            identb = small.tile([128, B], F32)
            nc.gpsimd.dma_start(out=identb, in_=id_d[:, :])
            ones_f = small.tile([128, 128], F32)
            nc.vector.memset(ones_f, 1.0)
            ones_r = small.tile([128, 128], F32R)
            nc.vector.tensor_copy(ones_r, ones_f)
            ones1_f = small.tile([1, 128], F32)
            nc.vector.memset(ones1_f, 1.0)
            ones1_r = small.tile([1, 128], F32R)
            nc.vector.tensor_copy(ones1_r, ones1_f)
            ln8neg = small.tile([128, 1], F32)
            nc.vector.memset(ln8neg, -float(np.log(8.0)))
            relu_bias = small.tile([128, 1], F32)
            nc.vector.memset(relu_bias, -0.999e4)
            half = small.tile([1, 1], F32)
            nc.vector.memset(half, 0.5)

            for _rep in range(repeat):
                # ---------- normalize: ssq via ones-matmul; rs = exp(-0.5 ln(ssq)) ----------
                def normalize(src, n, sq_engine, mul_engine, nm):
                    dst = big.tile([128, KC, n], F32R, tag=f"norm_{nm}", bufs=1)
                    if nm == "w":
                        ssq = mmp.tile([128, n], F32, tag="mm", name=f"ssq_{nm}")
                    else:
                        ssq = auxp.tile([128, n], F32, tag="aux", name=f"ssq_{nm}")
                    for k in range(KC):
                        sq = work.tile([128, n], F32R, tag="sq", name=f"sq_{nm}_{k}")
                        sq_engine(out=sq, in0=src[:, k, :], in1=src[:, k, :])
                        for n0 in range(0, n, 512):
                            ns = min(512, n - n0)
                            nc.tensor.matmul(ssq[:, n0:n0 + ns], ones_r, sq[:, n0:n0 + ns],
                                             start=(k == 0), stop=(k == KC - 1))
                    lns = work.tile([128, n], F32, tag="rs", name=f"lns_{nm}")
                    nc.scalar.activation(out=lns, in_=ssq, func=AF.Ln)
                    rs = work.tile([128, n], F32, tag="rs", name=f"rs_{nm}")
                    nc.scalar.activation(out=rs, in_=lns, func=AF.Exp, scale=-0.5)
                    _rs_cache[nm] = rs
                    for k in range(KC):
                        eng = gps_mul if k % 2 == 0 else mul_engine
                        eng(out=dst[:, k, :], in0=src[:, k, :], in1=rs)
                    return dst
                _rs_cache = {}

                def dve_mul(out, in0, in1):
                    nc.vector.tensor_mul(out, in0, in1)

                def gps_mul(out, in0, in1):
                    nc.gpsimd.tensor_mul(out, in0, in1)

                def act_sq(out, in0, in1):
                    nc.scalar.activation(out=out, in_=in0, func=AF.Square)

                wTn = normalize(wT_sb, CL, dve_mul, dve_mul, "w")
                # raw x and wl: cast to f32r; 1/||.|| folded into per-partition scales
                xT_r8 = big.tile([128, KC, B], F32R, tag="norm_x", bufs=1)
                wlT_r8 = big.tile([128, KC, B], F32R, tag="norm_wl", bufs=1)
                for k in range(KC):
                    nc.scalar.copy(xT_r8[:, k, :], xT_sb[:, k, :])
                    nc.gpsimd.tensor_copy(wlT_r8[:, k, :], wlT_sb[:, k, :])
                # ssq diag chains -> merged rsqrt [128, 16] -> a_all, al_all
                sdiag = small.tile([128, 2 * BT], F32)
                for nm2, src2, sq_fn, dcol in (("x", xT_sb, dve_mul, 0), ("wl", wlT_sb, None, BT)):
                    if nm2 == "wl":
                        ssq2 = mmp.tile([128, B], F32, tag="mm", name=f"ssq_{nm2}")
                    else:
                        ssq2 = auxp.tile([128, B], F32, tag="aux", name=f"ssq_{nm2}")
                    for k in range(KC):
                        sq2 = work.tile([128, B], F32R, tag="sq", name=f"sq_{nm2}_{k}")
                        if sq_fn is None:
                            nc.scalar.activation(out=sq2, in_=src2[:, k, :], func=AF.Square)
                        else:
                            sq_fn(out=sq2, in0=src2[:, k, :], in1=src2[:, k, :])
                        for n0 in range(0, B, 512):
                            nc.tensor.matmul(ssq2[:, n0:n0 + 512], ones_r, sq2[:, n0:n0 + 512],
                                             start=(k == 0), stop=(k == KC - 1))
                    dg2 = work.tile([128, B], F32, tag="prod", name=f"dg_{nm2}")
                    nc.vector.tensor_mul(dg2, ssq2, identb)
                    nc.vector.tensor_reduce(out=sdiag[:, dcol:dcol + BT],
                                            in_=dg2.rearrange("p (t i) -> p t i", i=128),
                                            axis=AXX, op=OP.add)
                sd_ln = small.tile([128, 2 * BT], F32)
                nc.scalar.activation(out=sd_ln, in_=sdiag, func=AF.Ln)
                rsq16 = small.tile([128, 2 * BT], F32)
                nc.scalar.activation(out=rsq16, in_=sd_ln, func=AF.Exp, scale=-0.5)
                a_all = rsq16[:, 0:BT]
                al_all = rsq16[:, BT:2 * BT]
                ga_all = small.tile([128, BT], F32)
                nc.vector.tensor_scalar_mul(ga_all, a_all, GAMMA)
                al14_all = small.tile([128, BT], F32)
                nc.vector.tensor_scalar_mul(al14_all, al_all, 1e4)

                # ---------- sp (raw rowdot, scaled by a*al afterwards) ----------
                sp_ps = auxp.tile([128, B], F32, tag="aux", name="sp_ps")
                for k in range(KC):
                    prod = work.tile([128, B], F32R, tag="prod")
                    nc.gpsimd.tensor_mul(prod, xT_sb[:, k, :], wlT_sb[:, k, :])
                    for n0 in range(0, B, 512):
                        nc.tensor.matmul(sp_ps[:, n0:n0 + 512], ones_r, prod[:, n0:n0 + 512],
                                         start=(k == 0), stop=(k == KC - 1))
                dgm = work.tile([128, B], F32, tag="prod", name="dgm")
                nc.vector.tensor_mul(dgm, sp_ps, identb)
                bias_all = small.tile([128, BT], F32)
                nc.vector.tensor_reduce(out=bias_all,
                                        in_=dgm.rearrange("p (t i) -> p t i", i=128),
                                        axis=AXX, op=OP.add)
                nc.vector.tensor_mul(bias_all, bias_all, a_all)
                nc.vector.tensor_mul(bias_all, bias_all, al_all)
                nc.vector.tensor_scalar_mul(bias_all, bias_all, -GAMMA)

                corr8 = small.tile([128, BT], F32)
                nc.scalar.activation(out=corr8, in_=cwl_pm, func=AF.Exp,
                                     scale=GAMMA * BETA, bias=ln8neg)

                # ---------- kexp = exp(G*B*cw) broadcast [128, CL] ----------
                kexp_row = small.tile([1, CL], F32R)
                nc.scalar.activation(out=kexp_row, in_=cw_row, func=AF.Exp, scale=GAMMA * BETA)
                kexp_ps = mmp.tile([128, CL], F32, tag="mm", name="kexp_ps")
                for n0 in range(0, CL, 512):
                    ns = min(512, CL - n0)
                    nc.tensor.matmul(kexp_ps[:, n0:n0 + ns], ones1_r, kexp_row[:, n0:n0 + ns],
                                     start=True, stop=True)
                kexp = big.tile([128, CL], mybir.dt.bfloat16, tag="kexp", bufs=1)
                nc.scalar.copy(kexp, kexp_ps)

                # ---------- main loop over batch tiles ----------
                HT = BT // 2
                S_h = [small.tile([128, HT], F32, name=f"S_h{h}") for h in range(2)]
                mx_h = [small.tile([128, HT], F32, name=f"mx_h{h}") for h in range(2)]
                for t in range(BT):
                    bsl = slice(t * 128, (t + 1) * 128)
                    pf = mmp.tile([128, CL], F32, tag="mm", name=f"pf{t}")
                    for k in range(KC):
                        for (n0, ns) in NCHUNKS:
                            nc.tensor.matmul(pf[:, n0:n0 + ns], xT_r8[:, k, bsl],
                                             wTn[:, k, n0:n0 + ns],
                                             start=(k == 0), stop=(k == KC - 1))
                    fea_sb = work.tile([128, CL], F32, tag="fea")
                    nc.scalar.activation(out=fea_sb, in_=pf, func=AF.Copy, scale=a_all[:, t:t + 1])
                    nc.sync.dma_start(out=fea_d[bsl, :], in_=fea_sb)
                    # scores
                    ps_ = mmp.tile([128, CL], F32, tag="mm", name=f"ps{t}")
                    for k in range(KC):
                        for (n0, ns) in NCHUNKS:
                            nc.tensor.matmul(ps_[:, n0:n0 + ns], wlT_r8[:, k, bsl],
                                             wTn[:, k, n0:n0 + ns],
                                             start=(k == 0), stop=(k == KC - 1))
                    r2 = work.tile([128, CL], F32, tag="r2", bufs=2)
                    nc.scalar.activation(out=r2, in_=ps_, func=AF.Relu, scale=al14_all[:, t:t + 1], bias=relu_bias)
                    msk = work.tile([128, CL], mybir.dt.bfloat16, tag="msk", bufs=3)
                    nc.vector.scalar_tensor_tensor(out=msk, in0=r2, scalar=-1e9, in1=ps_,
                                                   op0=OP.mult, op1=OP.add)
                    nc.vector.tensor_reduce(out=mx_h[t // 4][:, t % 4:t % 4 + 1], in_=msk,
                                            axis=AXX, op=OP.max)
                    E = work.tile([128, CL], mybir.dt.bfloat16, tag="E", bufs=3)
                    nc.scalar.activation(out=E, in_=pf, func=AF.Exp,
                                         bias=bias_all[:, t:t + 1], scale=ga_all[:, t:t + 1])
                    Ek = work.tile([128, CL], mybir.dt.bfloat16, tag="Ek", bufs=2)
                    nc.vector.scalar_tensor_tensor(out=Ek, in0=E, scalar=1.0, in1=kexp,
                                                   op0=OP.mult, op1=OP.mult,
                                                   accum_out=S_h[t // 4][:, t % 4:t % 4 + 1])

                # ---------- split collectives: one AllGather per half ----------
                if _rep == repeat - 1:
                    HB = HT * 128
                    red_halves = []
                    for h, (ci, co) in enumerate(((cc_in1, cc_out1), (cc_in2, cc_out2))):
                        hsl = slice(h * HT, (h + 1) * HT)
                        nc.vector.tensor_mul(mx_h[h], mx_h[h], al_all[:, hsl])
                        S_adj = small.tile([128, HT], F32, name=f"S_adj{h}")
                        nc.vector.tensor_sub(S_adj, S_h[h], corr8[:, hsl])
                        # interleaved pack: q = 2t+kind so the gathered buffer is one
                        # mergeable 2-D DMA: dram idx = (r*8 + 2t + k)*128 + p
                        ci_q = ci.rearrange("(q p) -> p q", p=128)
                        nc.sync.dma_start(out=ci_q[:, 0::2], in_=S_adj)
                        nc.gpsimd.dma_start(out=ci_q[:, 1::2], in_=mx_h[h])
                        nc.gpsimd.collective_compute(
                            "AllGather", OP.bypass,
                            ins=[ci[:]], outs=[co[:, :]],
                            replica_groups=[CORES],
                        )
                        g = small.tile([128, 8 * HT * 2], F32, name=f"g{h}")
                        nc.sync.dma_start(out=g, in_=co.rearrange("r (q p) -> p (r q)", p=128))
                        gv = g.rearrange("p (r t k) -> p t k r", r=8, t=HT)
                        addred = small.tile([128, HT, 2], F32, name=f"addred{h}")
                        nc.vector.tensor_reduce(out=addred, in_=gv, axis=AXX, op=OP.add)
                        maxred = small.tile([128, HT, 2], F32, name=f"maxred{h}")
                        nc.vector.tensor_reduce(out=maxred, in_=gv, axis=AXX, op=OP.max)
                        S_tot = addred[:, :, 0]
                        mx_tot = maxred[:, :, 1]
                        loss1 = small.tile([128, HT], F32, name=f"loss1{h}")
                        nc.scalar.activation(out=loss1, in_=S_tot, func=AF.Ln, bias=1.0)
                        redh = small.tile([128, 2], F32, name=f"redh{h}")
                        nc.vector.tensor_reduce(out=redh[:, 0:1], in_=loss1, axis=AXX, op=OP.add)
                        nc.vector.tensor_reduce(out=redh[:, 1:2], in_=mx_tot, axis=AXX, op=OP.add)
                        red_halves.append(redh)
                    red_r = small.tile([128, 2], F32R)
                    nc.vector.tensor_add(red_r, red_halves[0], red_halves[1])
                    pscal = mmp.tile([1, 2], F32, tag="mm", name="pscal")
                    nc.tensor.matmul(pscal, ones_r[:, 0:1], red_r, start=True, stop=True)
                    red2 = small.tile([1, 2], F32)
                    nc.scalar.copy(red2, pscal)
                    e1 = small.tile([1, 1], F32)
                    nc.scalar.activation(out=e1, in_=red2[0:1, 1:2], func=AF.Exp, scale=1.0 / B, bias=half)
                    lw = small.tile([1, 1], F32)
                    nc.scalar.activation(out=lw, in_=e1, func=AF.Ln, bias=1.0)
                    lw10 = small.tile([1, 1], F32)
                    nc.scalar.mul(lw10, lw, float(LAMDA))
                    loss_sb = small.tile([1, 1], F32)
                    nc.vector.scalar_tensor_tensor(out=loss_sb, in0=red2[0:1, 0:1], scalar=1.0 / B,
                                                   in1=lw10, op0=OP.mult, op1=OP.add)
                    nc.sync.dma_start(out=loss_d[:, :], in_=loss_sb)
    nc.finalize()
    return nc


_NC_CACHE = {}


def _get_nc():
    if "nc" not in _NC_CACHE:
        _NC_CACHE["nc"] = _build()
    return _NC_CACHE["nc"]


def _make_in_maps(input, weight, class_weight, label):
    x = np.ascontiguousarray(np.asarray(input, dtype=np.float32))
    w = np.ascontiguousarray(np.asarray(weight, dtype=np.float32))
    cw = np.ascontiguousarray(np.asarray(class_weight, dtype=np.float32))
    lab = np.asarray(label).astype(np.int64)

    wpad = np.concatenate([w, np.broadcast_to(w[0:1], (CP - C, D))], axis=0)
    cwpad = np.concatenate([cw, np.full(CP - C, -1e4, dtype=np.float32)])

    xT = np.ascontiguousarray(x.T)
    wlT = np.ascontiguousarray(w[lab].T)
    cwl = np.ascontiguousarray(cw[lab])
    identb = np.tile(np.eye(128, dtype=np.float32), (1, BT))

    in_maps = []
    for c in CORES:
        sl = slice(c * CL, (c + 1) * CL)
        in_maps.append({
            "xT": xT,
            "wT": np.ascontiguousarray(wpad[sl].T),
            "wlT": wlT,
            "cw": cwpad[sl].reshape(1, CL),
            "cwl": cwl,
            "identb": identb,
        })
    return in_maps


def run(input, weight, class_weight, label, trace=False):
    in_maps = _make_in_maps(input, weight, class_weight, label)
    nc = _get_nc()
    res = run_bass_kernel_spmd(nc, in_maps, CORES, trace=trace)
    fea = np.concatenate([res.results[c]["fea"] for c in CORES], axis=1)[:, :C]
    loss = np.float32(res.results[0]["loss"][0, 0])
    return (fea, loss), res


def kernel(input, weight, class_weight, label):
    (fea, loss), _ = run(input, weight, class_weight, label)
    return fea, loss
